# revision 13
# baseline (speedup 1.0000x reference)
"""AttentiveFP (2-layer GNN + GRU readout) as a Bass/Tile kernel on 8 TRN2 cores.

Strategy (data-parallel over the graph batch):
  - Graphs (contiguous node segments, graph_ids sorted) are split into 8
    chunks balanced by node count; each core owns its chunk's nodes.
  - Edges are assigned to the core owning their dst node, sorted by dst, and
    padded per 128-node destination block to multiples of 128 (dummy edges
    carry an out-of-range dst sentinel so indicator matrices zero them out).
  - Segment softmax+sum over dst is computed per 128-edge tile as
    U = (e*Ind)^T @ [1|he1] on the TensorEngine, where Ind[e,j] = (dst_e==j)
    is built with a vector-engine is_equal against a constant iota matrix.
    1/sum normalization, linear layers + GRUCell run per 128-node block.
  - Between the two GNN layers, each core's [p2b|h] rows are AllGathered so
    layer 2 can gather h[src] rows (src is global) by indirect DMA.
  - Readout: each core owns <=125 non-empty graphs (rank-local ids); the
    same indicator machinery reduces nodes->graphs; 2 GRU timesteps; final
    linear produces [128, 128] per core; host scatters rows to [1000, 128].
"""
import sys

if "/opt/trn_rl_repo" not in sys.path:
    sys.path.insert(0, "/opt/trn_rl_repo")

import numpy as np
import concourse.bass as bass
import concourse.tile as tile
from concourse import mybir, bacc, bass_utils

P = 128
W = 8
F32 = mybir.dt.float32
I32 = mybir.dt.int32
AOT = mybir.AluOpType
ACTF = mybir.ActivationFunctionType
SENT = 999.0  # dst-sentinel for dummy edges / padded nodes
EPS = 1e-20
OCC_THRESH = 1e-10


# ----------------------------------------------------------------------------
# host-side preprocessing
# ----------------------------------------------------------------------------

class Plan:
    pass


def preprocess(node_feats, edge_feats, src, dst, graph_ids, B):
    N, F = node_feats.shape
    M, E = edge_feats.shape
    pl = Plan()
    pl.N, pl.F, pl.M, pl.E, pl.B = N, F, M, E, B

    gids = np.asarray(graph_ids)
    counts = np.bincount(gids, minlength=B)
    ne_ids = np.nonzero(counts)[0]
    chunks = np.array_split(ne_ids, W)
    # node range per core
    node_starts, node_counts = [], []
    for ch in chunks:
        if len(ch) == 0:
            node_starts.append(N)
            node_counts.append(0)
            continue
        s = int(np.searchsorted(gids, ch[0], "left"))
        e = int(np.searchsorted(gids, ch[-1], "right"))
        node_starts.append(s)
        node_counts.append(e - s)
    pl.chunks = chunks
    node_starts = np.array(node_starts, np.int64)
    node_counts = np.array(node_counts, np.int64)
    pl.node_starts = node_starts
    pl.node_counts = node_counts

    n_max = max(1, int(max(node_counts)))
    NLOC = -(-n_max // P) * P
    NBLK = NLOC // P
    pl.NLOC, pl.NBLK = NLOC, NBLK

    # per-core rank (graph index within core) for each local node
    rankcol = np.full((W, P, NBLK), SENT, np.float32)
    for k in range(W):
        ch = chunks[k]
        nk = node_counts[k]
        if nk == 0:
            continue
        g_local = gids[node_starts[k]:node_starts[k] + nk]
        # map graph id -> rank within chunk
        rk = np.searchsorted(ch, g_local)
        r = np.full(NLOC, SENT, np.float32)
        r[:nk] = rk.astype(np.float32)
        rankcol[k] = r.reshape(NBLK, P).T  # [P, NBLK]
    pl.rankcol = rankcol

    # local node feats
    nf_loc = np.zeros((W, NLOC, F), np.float32)
    for k in range(W):
        nk = node_counts[k]
        nf_loc[k, :nk] = node_feats[node_starts[k]:node_starts[k] + nk]
    pl.nf_loc = nf_loc

    # edges by dst owner
    owner = np.searchsorted(node_starts, dst, "right") - 1
    per_core = []
    blk_counts = np.zeros((W, NBLK), np.int64)
    for k in range(W):
        sel = np.nonzero(owner == k)[0]
        dloc = dst[sel] - node_starts[k]
        order = np.argsort(dloc, kind="stable")
        sel = sel[order]
        dloc = dloc[order]
        per_core.append((sel, dloc))
        bc = np.bincount(dloc // P, minlength=NBLK)
        blk_counts[k] = bc[:NBLK]
    TB = np.maximum(1, -(-blk_counts.max(0) // P)).astype(np.int64)  # tiles/blk
    pl.TB = TB
    TT = int(TB.sum())
    pl.TT = TT

    srcg = np.zeros((W, TT, P), np.int32)
    hexti = np.zeros((W, TT, P), np.int32)
    dstmod = np.full((W, TT, P), SENT, np.float32)
    ef_pad = np.zeros((W, TT, P, E), np.float32)
    tile_off = np.concatenate([[0], np.cumsum(TB)])[:-1]  # block -> first tile
    pl.tile_off = tile_off
    src_owner = np.searchsorted(node_starts, src, "right") - 1
    hext_row_of_src = (src_owner * NLOC + (src - node_starts[src_owner])).astype(
        np.int32)
    for k in range(W):
        sel, dloc = per_core[k]
        blk = dloc // P
        # position within block
        for b in range(NBLK):
            m = blk == b
            cnt = int(m.sum())
            if cnt == 0:
                continue
            es = sel[m]
            t0 = tile_off[b]
            flat = np.arange(cnt)
            t_idx = t0 + flat // P
            p_idx = flat % P
            srcg[k, t_idx, p_idx] = src[es]
            hexti[k, t_idx, p_idx] = hext_row_of_src[es]
            dstmod[k, t_idx, p_idx] = (dloc[m] % P).astype(np.float32)
            ef_pad[k, t_idx, p_idx] = edge_feats[es]
    pl.srcg, pl.hexti, pl.dstmod, pl.ef_pad = srcg, hexti, dstmod, ef_pad
    return pl


# ----------------------------------------------------------------------------
# kernel builder
# ----------------------------------------------------------------------------

class Ctx:
    pass


def _transpose(cx, src_ap, k, dtype=F32):
    """PE-transpose src_ap [P, k] -> SBUF [k, P]."""
    nc = cx.nc
    tps = cx.psum.tile([P, P], F32, space="PSUM", tag="tr", bufs=2,
                       name="tps")
    nc.tensor.transpose(out=tps[:k, :P], in_=src_ap, identity=cx.ident[:, :])
    tsb = cx.work.tile([P, P], dtype, tag="tsb", bufs=4, name="tsb")
    nc.vector.tensor_copy(tsb[:k, :P], tps[:k, :P])
    return tsb[:k, :P]


def _bcast_col(cx, col_ap):
    """[P,1] column -> [P,P] matrix whose every row is col^T."""
    nc = cx.nc
    tps = cx.psum.tile([P, P], F32, space="PSUM", tag="tr", bufs=2, name="tps")
    nc.tensor.transpose(out=tps[:1, :P], in_=col_ap, identity=cx.ident[:, :])
    prow = cx.work.tile([1, P], F32, tag="prow", bufs=2, name="prow")
    nc.vector.tensor_copy(prow[:, :], tps[:1, :P])
    bps = cx.psum.tile([P, P], F32, space="PSUM", tag="gps", bufs=3, name="bps")
    nc.tensor.matmul(out=bps[:, :], lhsT=cx.ones_row[:, :], rhs=prow[:, :],
                     start=True, stop=True)
    bsb = cx.work.tile([P, P], F32, tag="bsb", bufs=2, name="bsb")
    nc.vector.tensor_copy(bsb[:, :], bps[:, :])
    return bsb


def _leaky(cx, out_ap, in_ap, tmp_tag="lk"):
    nc = cx.nc
    shape = [in_ap.shape[0], in_ap.shape[1]]
    tmp = cx.work.tile([P, 256], F32, tag=tmp_tag, bufs=2, name="lktmp")
    t = tmp[:shape[0], :shape[1]]
    nc.vector.tensor_scalar_mul(t, in_ap, 0.01)
    nc.vector.tensor_tensor(out=out_ap, in0=in_ap, in1=t, op=AOT.max)


def _elu(cx, out_ap, in_ap):
    """out = elu(in); in may be PSUM."""
    nc = cx.nc
    n, m = in_ap.shape[0], in_ap.shape[1]
    mn = cx.work.tile([P, 256], F32, tag="elu1", bufs=2, name="elmn")[:n, :m]
    ex = cx.work.tile([P, 256], F32, tag="elu2", bufs=2, name="elex")[:n, :m]
    rl = cx.work.tile([P, 256], F32, tag="elu3", bufs=2, name="elrl")[:n, :m]
    nc.vector.tensor_scalar_min(mn, in_ap, 0.0)
    nc.scalar.activation(ex, mn, ACTF.Exp)
    nc.vector.tensor_scalar_max(rl, in_ap, 0.0)
    nc.vector.tensor_scalar_add(ex, ex, -1.0)
    nc.vector.tensor_tensor(out=out_ap, in0=ex, in1=rl, op=AOT.add)


def _sigmoid(cx, out_ap, in_ap):
    """out = sigmoid(in) = 0.5*tanh(0.5x)+0.5; in may be PSUM."""
    nc = cx.nc
    nc.scalar.activation(out_ap, in_ap, ACTF.Tanh, scale=0.5)
    nc.vector.tensor_scalar(out=out_ap, in0=out_ap, scalar1=1.0, scalar2=0.5,
                            op0=AOT.add, op1=AOT.mult)


def _gru(cx, x_ap, h_ap, wname, out_ap, relu):
    """GRUCell for one 128-row block. x,h: [P,200] SBUF. out_ap: [P,200]."""
    nc = cx.nc
    G = cx.G
    wih0, wih1 = cx.consts[wname + "_wih0"], cx.consts[wname + "_wih1"]
    whh0, whh1 = cx.consts[wname + "_whh0"], cx.consts[wname + "_whh1"]
    brz = cx.consts[wname + "_brz"]
    bihn = cx.consts[wname + "_bihn"]
    bhhn = cx.consts[wname + "_bhhn"]
    H = G // 2
    xt0 = _transpose(cx, x_ap[:, 0:H], H)
    xt1 = _transpose(cx, x_ap[:, H:G], H)
    ht0 = _transpose(cx, h_ap[:, 0:H], H)
    ht1 = _transpose(cx, h_ap[:, H:G], H)

    a = cx.psum.tile([P, 2 * G], F32, space="PSUM", tag="gps", bufs=3,
                     name="gruA")
    nc.tensor.matmul(out=a[:, :], lhsT=xt0, rhs=wih0[:, 0:2 * G], start=True,
                     stop=False)
    nc.tensor.matmul(out=a[:, :], lhsT=xt1, rhs=wih1[:, 0:2 * G], start=False,
                     stop=False)
    nc.tensor.matmul(out=a[:, :], lhsT=ht0, rhs=whh0[:, 0:2 * G], start=False,
                     stop=False)
    nc.tensor.matmul(out=a[:, :], lhsT=ht1, rhs=whh1[:, 0:2 * G], start=False,
                     stop=False)
    nc.tensor.matmul(out=a[:, :], lhsT=cx.ones_row[:, :], rhs=brz[:, :],
                     start=False, stop=True)
    xn = cx.psum.tile([P, 2 * G], F32, space="PSUM", tag="gps", bufs=3,
                      name="gruXN")
    nc.tensor.matmul(out=xn[:, 0:G], lhsT=xt0, rhs=wih0[:, 2 * G:3 * G],
                     start=True, stop=False)
    nc.tensor.matmul(out=xn[:, 0:G], lhsT=xt1, rhs=wih1[:, 2 * G:3 * G],
                     start=False, stop=False)
    nc.tensor.matmul(out=xn[:, 0:G], lhsT=cx.ones_row[:, :], rhs=bihn[:, :],
                     start=False, stop=True)
    hn = cx.psum.tile([P, 2 * G], F32, space="PSUM", tag="gps", bufs=3,
                      name="gruHN")
    nc.tensor.matmul(out=hn[:, 0:G], lhsT=ht0, rhs=whh0[:, 2 * G:3 * G],
                     start=True, stop=False)
    nc.tensor.matmul(out=hn[:, 0:G], lhsT=ht1, rhs=whh1[:, 2 * G:3 * G],
                     start=False, stop=False)
    nc.tensor.matmul(out=hn[:, 0:G], lhsT=cx.ones_row[:, :], rhs=bhhn[:, :],
                     start=False, stop=True)

    rz = cx.work.tile([P, 2 * G], F32, tag="rz", bufs=2, name="rz")
    _sigmoid(cx, rz[:, :], a[:, :])
    t1 = cx.work.tile([P, G], F32, tag="gt1", bufs=2, name="gt1")
    nc.vector.tensor_tensor(out=t1[:, :], in0=rz[:, 0:G], in1=hn[:, 0:G],
                            op=AOT.mult)
    t2 = cx.work.tile([P, G], F32, tag="gt2", bufs=2, name="gt2")
    nc.vector.tensor_tensor(out=t2[:, :], in0=t1[:, :], in1=xn[:, 0:G],
                            op=AOT.add)
    n_ = cx.work.tile([P, G], F32, tag="gn", bufs=2, name="gn")
    nc.scalar.activation(n_[:, :], t2[:, :], ACTF.Tanh)
    d = cx.work.tile([P, G], F32, tag="gd", bufs=2, name="gd")
    nc.vector.tensor_tensor(out=d[:, :], in0=h_ap, in1=n_[:, :], op=AOT.subtract)
    e2 = cx.work.tile([P, G], F32, tag="ge", bufs=2, name="ge")
    nc.vector.tensor_tensor(out=e2[:, :], in0=rz[:, G:2 * G], in1=d[:, :],
                            op=AOT.mult)
    if relu:
        hn2 = cx.work.tile([P, G], F32, tag="gh", bufs=2, name="gh")
        nc.vector.tensor_tensor(out=hn2[:, :], in0=n_[:, :], in1=e2[:, :],
                                op=AOT.add)
        nc.vector.tensor_scalar_max(out_ap, hn2[:, :], 0.0)
    else:
        nc.vector.tensor_tensor(out=out_ap, in0=n_[:, :], in1=e2[:, :],
                                op=AOT.add)


def _ttr(cx, in0, in1, scalar, accum_out, width):
    """accum_out[p] = sum_f(in0*in1) + scalar.  (tensor_tensor_reduce is
    broken on HW, so this is mult + reduce + add.)"""
    nc = cx.nc
    scr = cx.work.tile([P, 256], F32, tag="ttrscr", bufs=2, name="ttrscr")
    s = scr[:, 0:width]
    nc.vector.tensor_tensor(out=s, in0=in0, in1=in1, op=AOT.mult)
    nc.vector.reduce_sum(accum_out, s, axis=mybir.AxisListType.X)
    if isinstance(scalar, float):
        if scalar != 0.0:
            nc.vector.tensor_scalar_add(accum_out, accum_out, scalar)
    else:
        nc.vector.tensor_tensor(out=accum_out, in0=accum_out, in1=scalar,
                                op=AOT.add)


def build_program(pl, weights, debug=False):
    """Build the full Bass program. weights: dict of numpy arrays."""
    G = 200
    PD = weights["t_w"].shape[1]
    NBLK, TT, TB = pl.NBLK, pl.TT, pl.TB
    NLOC = pl.NLOC
    E, F = pl.E, pl.F
    T_steps = weights["r_proj_w"].shape[0]

    nc = bacc.Bacc("TRN2", num_devices=W, debug=False)
    cx = Ctx()
    cx.nc = nc
    cx.G = G

    # ---- external inputs -------------------------------------------------
    def inp(name, shape, dt=F32):
        return nc.dram_tensor(name, list(shape), dt, kind="ExternalInput")

    t_nf_full = inp("node_feats", [pl.N, F])
    t_nf_loc = inp("nf_loc", [NLOC, F])
    t_ef = inp("ef_pad", [TT, P, E])
    t_srcg = inp("srcg", [TT, P], I32)
    t_hexti = inp("hexti", [TT, P], I32)
    t_dstmod = inp("dstmod", [TT, P])
    t_rankcol = inp("rankcol", [P, NBLK])
    t_iota = inp("iota_bc", [P, P])
    t_ident = inp("ident", [P, P])
    t_ones_row = inp("ones_row", [1, P])
    t_ones_col = inp("ones_col", [P, 1])
    w_in = {}
    wspec = {
        "pnw_ext": [F + 1, G], "pe1w_ext": [F + E + 1, G],
        "w2a_bc": [P, G], "w2b_bc": [P, G],
        "w2a2_bc": [P, G], "w2b2_bc": [P, G],
        "etw": [G, G], "etb_row": [1, G],
        "lpnw": [G, G], "lpnb_row": [1, G],
        "tw": [G, PD], "tb_row": [1, PD],
    }
    for t in range(T_steps):
        wspec[f"wA{t}_bc"] = [P, G]
        wspec[f"wB{t}_bc"] = [P, G]
        wspec[f"rpw{t}"] = [G, G]
        wspec[f"rpb{t}_row"] = [1, G]
    for nm in ["g0", "g1"] + [f"gr{t}" for t in range(T_steps)]:
        wspec[nm + "_wih"] = [G, 3 * G]
        wspec[nm + "_whh"] = [G, 3 * G]
        wspec[nm + "_brz"] = [1, 2 * G]
        wspec[nm + "_bihn"] = [1, G]
        wspec[nm + "_bhhn"] = [1, G]
    for name, shp in wspec.items():
        w_in[name] = inp("w_" + name, shp)

    t_out = nc.dram_tensor("out", [P, PD], F32, kind="ExternalOutput")
    dbg = {}
    if debug:
        for nm, shp in [("hv_dbg", [NLOC, G]), ("h1_dbg", [NLOC, G]),
                        ("h2_dbg", [NLOC, G]), ("g0_dbg", [P, G + 1]),
                        ("u1_dbg", [NLOC, G + 1]), ("p1_dbg", [NLOC, 1]),
                        ("gf_dbg", [P, G])]:
            dbg[nm] = nc.dram_tensor(nm, shp, F32, kind="ExternalOutput")

    TMAX = int(TB.max())

    with tile.TileContext(nc) as tc:
        with tc.tile_pool(name="const", bufs=1) as cpool, \
             tc.tile_pool(name="work", bufs=2) as work, \
             tc.tile_pool(name="psum", bufs=2, space="PSUM") as psum, \
             tc.tile_pool(name="dram", bufs=1, space="DRAM") as dram:
            cx.work, cx.psum = work, psum

            # ---- persistent consts --------------------------------------
            cx.consts = {}

            def load_const(name, src_ap, shape):
                t = cpool.tile(shape, F32, tag="c_" + name, name="c_" + name)
                nc.sync.dma_start(out=t[:, :], in_=src_ap)
                cx.consts[name] = t
                return t

            cx.ident = load_const("ident", t_ident[:, :], [P, P])
            cx.iota = load_const("iota", t_iota[:, :], [P, P])
            cx.ones_row = load_const("ones_row", t_ones_row[:, :], [1, P])
            cx.ones_col = load_const("ones_col", t_ones_col[:, :], [P, 1])
            load_const("pe1w_nf", w_in["pe1w_ext"][0:F, :], [F, G])
            load_const("pe1w_ef", w_in["pe1w_ext"][F:F + E, :], [E, G])
            load_const("pe1w_b", w_in["pe1w_ext"][F + E:F + E + 1, :], [1, G])
            for name, shp in wspec.items():
                if name == "pe1w_ext":
                    continue
                if name.endswith("_wih") or name.endswith("_whh"):
                    base = w_in[name]
                    load_const(name + "0", base[0:G // 2, :], [G // 2, shp[1]])
                    load_const(name + "1", base[G // 2:G, :], [G // 2, shp[1]])
                elif name in ("etw", "lpnw", "tw") or name.startswith("rpw"):
                    base = w_in[name]
                    load_const(name + "0", base[0:G // 2, :], [G // 2, shp[1]])
                    load_const(name + "1", base[G // 2:G, :], [G // 2, shp[1]])
                else:
                    load_const(name, w_in[name][:, :], shp)

            rankcol = load_const("rankcol", t_rankcol[:, :], [P, NBLK])
            p2a_all = cpool.tile([P, NBLK], F32, tag="p2a_all", name="p2a_all")

            hext_loc = dram.tile([NLOC, G + 2], F32, name="hext_loc")
            hext_glob = dram.tile([W * NLOC, G + 2], F32, name="hext_glob")

            hblk = [cpool.tile([P, G + 2], F32, tag=f"hblk{b}",
                               name=f"hblk{b}") for b in range(NBLK)]

            scalars = weights["scalars"]
            pe2_b = float(scalars["pe2_b"])
            lpe_b = float(scalars["lpe_b"])
            rlb = [float(x) for x in scalars["r_logit_b"]]

            # =============================================================
            # Layer 1 + GRU0, block by block
            # =============================================================
            for b in range(NBLK):
                tb = int(TB[b])
                t0 = int(pl.tile_off[b])
                # --- hv_new for this block -------------------------------
                nfx = work.tile([P, F + 1], F32, tag="nfx", bufs=2, name="nfx")
                nc.sync.dma_start(out=nfx[:, 0:F],
                                  in_=t_nf_loc[b * P:(b + 1) * P, :])
                nc.gpsimd.memset(nfx[:, F:F + 1], 1.0)
                nfxt = _transpose(cx, nfx[:, :], F + 1)
                hvps = psum.tile([P, G], F32, space="PSUM", tag="he1ps",
                                 bufs=1, name="hvps")
                nc.tensor.matmul(out=hvps[:, :], lhsT=nfxt,
                                 rhs=cx.consts["pnw_ext"][:, :], start=True,
                                 stop=True)
                hv = work.tile([P, G], F32, tag="hv", bufs=3, name="hv")
                _leaky(cx, hv[:, :], hvps[:, :])
                # p1 = hv @ pe2_w[:200] + pe2_b
                p1 = work.tile([P, 1], F32, tag="p1", bufs=2, name="p1")
                _ttr(cx, hv[:, :], cx.consts["w2a_bc"][:, :], pe2_b,
                     p1[:, 0:1], G)
                p_bc = _bcast_col(cx, p1[:, 0:1])
                if debug:
                    nc.sync.dma_start(out=dbg["hv_dbg"][b * P:(b + 1) * P, :],
                                      in_=hv[:, :])
                    nc.sync.dma_start(out=dbg["p1_dbg"][b * P:(b + 1) * P, :],
                                      in_=p1[:, :])

                # --- edge tiles: stage A ---------------------------------
                zst = work.tile([P, TMAX], F32, tag="zst", bufs=2, name="zst")
                inds = []
                he1s = []
                for t in range(tb):
                    gt = t0 + t
                    xg = work.tile([P, F], F32, tag="xg", bufs=4, name="xg")
                    idxt = work.tile([P, 1], I32, tag="idxt", bufs=4,
                                     name="idxt")
                    nc.sync.dma_start(
                        out=idxt[:, :],
                        in_=t_srcg[gt:gt + 1, :].rearrange("a p -> p a"))
                    nc.gpsimd.indirect_dma_start(
                        out=xg[:, :], out_offset=None, in_=t_nf_full[:, :],
                        in_offset=bass.IndirectOffsetOnAxis(ap=idxt[:, :],
                                                            axis=0))
                    eft = work.tile([P, E], F32, tag="eft", bufs=4, name="eft")
                    nc.sync.dma_start(out=eft[:, :], in_=t_ef[gt, :, :])
                    xgt = _transpose(cx, xg[:, :], F)
                    eftt = _transpose(cx, eft[:, :], E)
                    h1ps = psum.tile([P, G], F32, space="PSUM", tag="he1ps",
                                     bufs=1, name="h1ps")
                    nc.tensor.matmul(out=h1ps[:, :], lhsT=xgt,
                                     rhs=cx.consts["pe1w_nf"][:, :],
                                     start=True, stop=False)
                    nc.tensor.matmul(out=h1ps[:, :], lhsT=eftt,
                                     rhs=cx.consts["pe1w_ef"][:, :],
                                     start=False, stop=False)
                    nc.tensor.matmul(out=h1ps[:, :], lhsT=cx.ones_row[:, :],
                                     rhs=cx.consts["pe1w_b"][:, :],
                                     start=False, stop=True)
                    he1 = work.tile([P, G + 4], F32, tag="he1",
                                    bufs=TMAX + 1, name="he1")
                    nc.gpsimd.memset(he1[:, 0:1], 1.0)
                    _leaky(cx, he1[:, 1:G + 1], h1ps[:, :])
                    dmc = work.tile([P, 1], F32, tag="dmc", bufs=4, name="dmc")
                    nc.sync.dma_start(
                        out=dmc[:, :],
                        in_=t_dstmod[gt:gt + 1, :].rearrange("a p -> p a"))
                    ind = work.tile([P, P], F32, tag="ind", bufs=TMAX + 1,
                                    name="ind")
                    nc.vector.tensor_scalar(
                        out=ind[:, :], in0=cx.iota[:, :], scalar1=dmc[:, 0:1],
                        scalar2=None, op0=AOT.is_equal)
                    za = work.tile([P, 1], F32, tag="za", bufs=4, name="za")
                    _ttr(cx, he1[:, 1:G + 1], cx.consts["w2b_bc"][:, :], 0.0,
                         za[:, 0:1], G)
                    _ttr(cx, ind[:, :], p_bc[:, :], za[:, 0:1],
                         zst[:, t:t + 1], P)
                    inds.append(ind)
                    he1s.append(he1)
                # --- exp over the block ----------------------------------
                zl = work.tile([P, TMAX], F32, tag="zl", bufs=2, name="zl")
                _leaky(cx, zl[:, 0:tb], zst[:, 0:tb], tmp_tag="lkz")
                ee = work.tile([P, TMAX], F32, tag="ee", bufs=2, name="ee")
                nc.scalar.activation(ee[:, 0:tb], zl[:, 0:tb], ACTF.Exp)
                # --- stage B: weighted segment reduction -----------------
                u1 = psum.tile([P, G + 1], F32, space="PSUM", tag="U",
                               bufs=2, name="u1")
                for t in range(tb):
                    wt = work.tile([P, P], F32, tag="wt", bufs=3, name="wt")
                    nc.vector.tensor_scalar_mul(wt[:, :], inds[t][:, :],
                                                ee[:, t:t + 1])
                    nc.tensor.matmul(out=u1[:, :], lhsT=wt[:, :],
                                     rhs=he1s[t][:, 0:G + 1],
                                     start=(t == 0), stop=(t == tb - 1))
                if debug:
                    u1sb = work.tile([P, G + 1], F32, tag="u1sb", bufs=2,
                                     name="u1sb")
                    nc.vector.tensor_copy(u1sb[:, :], u1[:, :])
                    nc.sync.dma_start(out=dbg["u1_dbg"][b * P:(b + 1) * P, :],
                                      in_=u1sb[:, :])
                # --- normalize + linear + GRU0 ---------------------------
                s_sb = work.tile([P, 1], F32, tag="ssb", bufs=2, name="ssb")
                nc.vector.tensor_scalar_add(s_sb[:, :], u1[:, 0:1], EPS)
                rec = work.tile([P, 1], F32, tag="rec", bufs=2, name="rec")
                nc.vector.reciprocal(rec[:, :], s_sb[:, :])
                occ = work.tile([P, 1], F32, tag="occ", bufs=2, name="occ")
                nc.vector.tensor_scalar(out=occ[:, :], in0=s_sb[:, :],
                                        scalar1=OCC_THRESH, scalar2=None,
                                        op0=AOT.is_ge)
                csum = work.tile([P, G], F32, tag="csum", bufs=2, name="csum")
                nc.vector.tensor_scalar_mul(csum[:, :], u1[:, 1:G + 1],
                                            rec[:, 0:1])
                ct0 = _transpose(cx, csum[:, 0:G // 2], G // 2)
                ct1 = _transpose(cx, csum[:, G // 2:G], G // 2)
                occ_t = _transpose(cx, occ[:, 0:1], 1)
                cg = psum.tile([P, G], F32, space="PSUM", tag="gps", bufs=3,
                               name="cg")
                nc.tensor.matmul(out=cg[:, :], lhsT=ct0,
                                 rhs=cx.consts["etw0"][:, :], start=True,
                                 stop=False)
                nc.tensor.matmul(out=cg[:, :], lhsT=ct1,
                                 rhs=cx.consts["etw1"][:, :], start=False,
                                 stop=False)
                nc.tensor.matmul(out=cg[:, :], lhsT=occ_t,
                                 rhs=cx.consts["etb_row"][:, :], start=False,
                                 stop=True)
                x1 = work.tile([P, G], F32, tag="x1", bufs=2, name="x1")
                _elu(cx, x1[:, :], cg[:, :])
                _gru(cx, x1[:, :], hv[:, :], "g0", hblk[b][:, 1:G + 1],
                     relu=True)
                # p2a / p2b
                _ttr(cx, hblk[b][:, 1:G + 1], cx.consts["w2a2_bc"][:, :],
                     lpe_b, p2a_all[:, b:b + 1], G)
                _ttr(cx, hblk[b][:, 1:G + 1], cx.consts["w2b2_bc"][:, :], 0.0,
                     hblk[b][:, 0:1], G)
                nc.gpsimd.memset(hblk[b][:, G + 1:G + 2], 1.0)
                nc.sync.dma_start(out=hext_loc[b * P:(b + 1) * P, :],
                                  in_=hblk[b][:, 0:G + 2])
                if debug:
                    nc.sync.dma_start(out=dbg["h1_dbg"][b * P:(b + 1) * P, :],
                                      in_=hblk[b][:, 1:G + 1])

            # =============================================================
            # AllGather h_ext
            # =============================================================
            nc.gpsimd.collective_compute(
                "AllGather", AOT.bypass, replica_groups=[list(range(W))],
                ins=[hext_loc.opt()], outs=[hext_glob.opt()])

            # =============================================================
            # Layer 2 + GRU1, block by block
            # =============================================================
            for b in range(NBLK):
                tb = int(TB[b])
                t0 = int(pl.tile_off[b])
                p2_bc = _bcast_col(cx, p2a_all[:, b:b + 1])
                zst = work.tile([P, TMAX], F32, tag="zst", bufs=2, name="zst")
                inds = []
                etgs = []
                for t in range(tb):
                    gt = t0 + t
                    etg = work.tile([P, G + 2], F32, tag="etg",
                                    bufs=TMAX + 1, name="etg")
                    idxt = work.tile([P, 1], I32, tag="idxt", bufs=4,
                                     name="idxt")
                    nc.sync.dma_start(
                        out=idxt[:, :],
                        in_=t_hexti[gt:gt + 1, :].rearrange("a p -> p a"))
                    nc.gpsimd.indirect_dma_start(
                        out=etg[:, :], out_offset=None,
                        in_=hext_glob[:, :],
                        in_offset=bass.IndirectOffsetOnAxis(ap=idxt[:, :],
                                                            axis=0))
                    dmc = work.tile([P, 1], F32, tag="dmc", bufs=4, name="dmc")
                    nc.sync.dma_start(
                        out=dmc[:, :],
                        in_=t_dstmod[gt:gt + 1, :].rearrange("a p -> p a"))
                    ind = work.tile([P, P], F32, tag="ind", bufs=TMAX + 1,
                                    name="ind")
                    nc.vector.tensor_scalar(
                        out=ind[:, :], in0=cx.iota[:, :], scalar1=dmc[:, 0:1],
                        scalar2=None, op0=AOT.is_equal)
                    _ttr(cx, ind[:, :], p2_bc[:, :], etg[:, 0:1],
                         zst[:, t:t + 1], P)
                    inds.append(ind)
                    etgs.append(etg)
                zl = work.tile([P, TMAX], F32, tag="zl", bufs=2, name="zl")
                _leaky(cx, zl[:, 0:tb], zst[:, 0:tb], tmp_tag="lkz")
                ee = work.tile([P, TMAX], F32, tag="ee", bufs=2, name="ee")
                nc.scalar.activation(ee[:, 0:tb], zl[:, 0:tb], ACTF.Exp)
                u2 = psum.tile([P, G + 1], F32, space="PSUM", tag="U",
                               bufs=2, name="u2")
                for t in range(tb):
                    wt = work.tile([P, P], F32, tag="wt", bufs=3, name="wt")
                    nc.vector.tensor_scalar_mul(wt[:, :], inds[t][:, :],
                                                ee[:, t:t + 1])
                    nc.tensor.matmul(out=u2[:, :], lhsT=wt[:, :],
                                     rhs=etgs[t][:, 1:G + 2],
                                     start=(t == 0), stop=(t == tb - 1))
                s_sb = work.tile([P, 1], F32, tag="ssb", bufs=2, name="ssb")
                nc.vector.tensor_scalar_add(s_sb[:, :], u2[:, G:G + 1], EPS)
                rec = work.tile([P, 1], F32, tag="rec", bufs=2, name="rec")
                nc.vector.reciprocal(rec[:, :], s_sb[:, :])
                occ = work.tile([P, 1], F32, tag="occ", bufs=2, name="occ")
                nc.vector.tensor_scalar(out=occ[:, :], in0=s_sb[:, :],
                                        scalar1=OCC_THRESH, scalar2=None,
                                        op0=AOT.is_ge)
                csum = work.tile([P, G], F32, tag="csum", bufs=2, name="csum")
                nc.vector.tensor_scalar_mul(csum[:, :], u2[:, 0:G],
                                            rec[:, 0:1])
                ct0 = _transpose(cx, csum[:, 0:G // 2], G // 2)
                ct1 = _transpose(cx, csum[:, G // 2:G], G // 2)
                occ_t = _transpose(cx, occ[:, 0:1], 1)
                cg = psum.tile([P, G], F32, space="PSUM", tag="gps", bufs=3,
                               name="cg2")
                nc.tensor.matmul(out=cg[:, :], lhsT=ct0,
                                 rhs=cx.consts["lpnw0"][:, :], start=True,
                                 stop=False)
                nc.tensor.matmul(out=cg[:, :], lhsT=ct1,
                                 rhs=cx.consts["lpnw1"][:, :], start=False,
                                 stop=False)
                nc.tensor.matmul(out=cg[:, :], lhsT=occ_t,
                                 rhs=cx.consts["lpnb_row"][:, :], start=False,
                                 stop=True)
                x2 = work.tile([P, G], F32, tag="x1", bufs=2, name="x2")
                _elu(cx, x2[:, :], cg[:, :])
                _gru(cx, x2[:, :], hblk[b][:, 1:G + 1], "g1",
                     hblk[b][:, 1:G + 1], relu=True)
                if debug:
                    nc.sync.dma_start(out=dbg["h2_dbg"][b * P:(b + 1) * P, :],
                                      in_=hblk[b][:, 1:G + 1])

            # =============================================================
            # Readout
            # =============================================================
            def mk_indg(b):
                t = work.tile([P, P], F32, tag="indg", bufs=3, name="indg")
                nc.vector.tensor_scalar(
                    out=t[:, :], in0=cx.iota[:, :],
                    scalar1=rankcol[:, b:b + 1], scalar2=None,
                    op0=AOT.is_equal)
                return t

            g0ps = psum.tile([P, G + 1], F32, space="PSUM", tag="U", bufs=2,
                             name="g0ps")
            for b in range(NBLK):
                gi = mk_indg(b)
                nc.tensor.matmul(out=g0ps[:, :], lhsT=gi[:, :],
                                 rhs=hblk[b][:, 1:G + 2], start=(b == 0),
                                 stop=(b == NBLK - 1))
            g_sb = work.tile([P, G], F32, tag="gsb", bufs=2, name="gsb")
            nc.vector.tensor_copy(g_sb[:, :], g0ps[:, 0:G])
            occg = work.tile([P, 1], F32, tag="occg", bufs=1, name="occg")
            nc.vector.tensor_scalar(out=occg[:, :], in0=g0ps[:, G:G + 1],
                                    scalar1=0.5, scalar2=None, op0=AOT.is_ge)
            occg_t = _transpose(cx, occg[:, 0:1], 1)
            occg_tc = cpool.tile([1, P], F32, tag="occgt", name="occgt")
            nc.vector.tensor_copy(occg_tc[:, :], occg_t)
            if debug:
                g0sb = work.tile([P, G + 1], F32, tag="g0sb", bufs=1,
                                 name="g0sb")
                nc.vector.tensor_copy(g0sb[:, :], g0ps[:, :])
                nc.sync.dma_start(out=dbg["g0_dbg"][:, :], in_=g0sb[:, :])

            for ts in range(T_steps):
                grelu = work.tile([P, G], F32, tag="grelu", bufs=2,
                                  name="grelu")
                nc.vector.tensor_scalar_max(grelu[:, :], g_sb[:, :], 0.0)
                q1 = work.tile([P, 1], F32, tag="q1", bufs=2, name="q1")
                _ttr(cx, grelu[:, :], cx.consts[f"wA{ts}_bc"][:, :], rlb[ts],
                     q1[:, 0:1], G)
                q_bc = _bcast_col(cx, q1[:, 0:1])
                zstr = work.tile([P, NBLK], F32, tag="zstr", bufs=2,
                                 name="zstr")
                for b in range(NBLK):
                    za = work.tile([P, 1], F32, tag="za", bufs=4, name="za")
                    _ttr(cx, hblk[b][:, 1:G + 1], cx.consts[f"wB{ts}_bc"][:, :],
                         0.0, za[:, 0:1], G)
                    gi = mk_indg(b)
                    _ttr(cx, gi[:, :], q_bc[:, :], za[:, 0:1],
                         zstr[:, b:b + 1], P)
                zlr = work.tile([P, NBLK], F32, tag="zlr", bufs=2, name="zlr")
                _leaky(cx, zlr[:, :], zstr[:, :], tmp_tag="lkz")
                eer = work.tile([P, NBLK], F32, tag="eer", bufs=2, name="eer")
                nc.scalar.activation(eer[:, :], zlr[:, :], ACTF.Exp)
                ur = psum.tile([P, G + 1], F32, space="PSUM", tag="U", bufs=2,
                               name="ur")
                for b in range(NBLK):
                    gi = mk_indg(b)
                    wt = work.tile([P, P], F32, tag="wt", bufs=3, name="wt")
                    nc.vector.tensor_scalar_mul(wt[:, :], gi[:, :],
                                                eer[:, b:b + 1])
                    nc.tensor.matmul(out=ur[:, :], lhsT=wt[:, :],
                                     rhs=hblk[b][:, 1:G + 2], start=(b == 0),
                                     stop=(b == NBLK - 1))
                s_sb = work.tile([P, 1], F32, tag="ssb", bufs=2, name="ssb")
                nc.vector.tensor_scalar_add(s_sb[:, :], ur[:, G:G + 1], EPS)
                rec = work.tile([P, 1], F32, tag="rec", bufs=2, name="rec")
                nc.vector.reciprocal(rec[:, :], s_sb[:, :])
                gnum = work.tile([P, G], F32, tag="csum", bufs=2, name="gnum")
                nc.vector.tensor_scalar_mul(gnum[:, :], ur[:, 0:G],
                                            rec[:, 0:1])
                gt0 = _transpose(cx, gnum[:, 0:G // 2], G // 2)
                gt1 = _transpose(cx, gnum[:, G // 2:G], G // 2)
                gr = psum.tile([P, G], F32, space="PSUM", tag="gps", bufs=3,
                               name="gr")
                nc.tensor.matmul(out=gr[:, :], lhsT=gt0,
                                 rhs=cx.consts[f"rpw{ts}0"][:, :], start=True,
                                 stop=False)
                nc.tensor.matmul(out=gr[:, :], lhsT=gt1,
                                 rhs=cx.consts[f"rpw{ts}1"][:, :], start=False,
                                 stop=False)
                nc.tensor.matmul(out=gr[:, :], lhsT=occg_tc[:, :],
                                 rhs=cx.consts[f"rpb{ts}_row"][:, :],
                                 start=False, stop=True)
                xr = work.tile([P, G], F32, tag="x1", bufs=2, name="xr")
                _elu(cx, xr[:, :], gr[:, :])
                gnew = work.tile([P, G], F32, tag="gsb", bufs=2, name="gnew")
                _gru(cx, xr[:, :], g_sb[:, :], f"gr{ts}", gnew[:, :],
                     relu=False)
                g_sb = gnew

            if debug:
                nc.sync.dma_start(out=dbg["gf_dbg"][:, :], in_=g_sb[:, :])
            ft0 = _transpose(cx, g_sb[:, 0:G // 2], G // 2)
            ft1 = _transpose(cx, g_sb[:, G // 2:G], G // 2)
            ops = psum.tile([P, PD], F32, space="PSUM", tag="gps", bufs=3,
                            name="ops")
            nc.tensor.matmul(out=ops[:, :], lhsT=ft0,
                             rhs=cx.consts["tw0"][:, :], start=True,
                             stop=False)
            nc.tensor.matmul(out=ops[:, :], lhsT=ft1,
                             rhs=cx.consts["tw1"][:, :], start=False,
                             stop=False)
            nc.tensor.matmul(out=ops[:, :], lhsT=cx.ones_row[:, :],
                             rhs=cx.consts["tb_row"][:, :], start=False,
                             stop=True)
            osb = work.tile([P, PD], F32, tag="osb", bufs=1, name="osb")
            nc.vector.tensor_copy(osb[:, :], ops[:, :])
            nc.sync.dma_start(out=t_out[:, :], in_=osb[:, :])

    nc.compile()
    return nc


# ----------------------------------------------------------------------------
# weight packing (shared by all cores)
# ----------------------------------------------------------------------------

def pack_weights(i, F, E, G, PD):
    def bc(row):
        return np.tile(np.asarray(row, np.float32).reshape(1, -1), (P, 1))

    T_steps = i["r_proj_w"].shape[0]
    wp = {}
    wp["pnw_ext"] = np.concatenate([i["pn_w"], i["pn_b"][None, :]], 0)
    wp["pe1w_ext"] = np.concatenate([i["pe1_w"], i["pe1_b"][None, :]], 0)
    wp["w2a_bc"] = bc(i["pe2_w"][0:G, 0])
    wp["w2b_bc"] = bc(i["pe2_w"][G:2 * G, 0])
    wp["w2a2_bc"] = bc(i["lpe_w"][0:G, 0])
    wp["w2b2_bc"] = bc(i["lpe_w"][G:2 * G, 0])
    wp["etw"] = i["et_w"]
    wp["etb_row"] = i["et_b"][None, :]
    wp["lpnw"] = i["lpn_w"]
    wp["lpnb_row"] = i["lpn_b"][None, :]
    wp["tw"] = i["t_w"]
    wp["tb_row"] = i["t_b"][None, :]
    for t in range(T_steps):
        wp[f"wA{t}_bc"] = bc(i["r_logit_w"][t, 0:G, 0])
        wp[f"wB{t}_bc"] = bc(i["r_logit_w"][t, G:2 * G, 0])
        wp[f"rpw{t}"] = i["r_proj_w"][t]
        wp[f"rpb{t}_row"] = i["r_proj_b"][t][None, :]
    packs = [("g0", i["gru0_wih"], i["gru0_whh"], i["gru0_bih"], i["gru0_bhh"]),
             ("g1", i["gru1_wih"], i["gru1_whh"], i["gru1_bih"], i["gru1_bhh"])]
    for t in range(T_steps):
        packs.append((f"gr{t}", i["r_gru_wih"][t], i["r_gru_whh"][t],
                      i["r_gru_bih"][t], i["r_gru_bhh"][t]))
    for nm, wih, whh, bih, bhh in packs:
        wp[nm + "_wih"] = wih
        wp[nm + "_whh"] = whh
        wp[nm + "_brz"] = (bih[0:2 * G] + bhh[0:2 * G])[None, :]
        wp[nm + "_bihn"] = bih[2 * G:3 * G][None, :]
        wp[nm + "_bhhn"] = bhh[2 * G:3 * G][None, :]
    wp = {k: np.ascontiguousarray(v, np.float32) for k, v in wp.items()}
    wp["r_proj_w"] = i["r_proj_w"]
    wp["scalars"] = dict(pe2_b=float(i["pe2_b"][0]),
                         lpe_b=float(i["lpe_b"][0]),
                         r_logit_b=[float(x) for x in i["r_logit_b"][:, 0]])
    wp["t_w"] = i["t_w"]
    return wp


def make_in_maps(pl, inputs, wp):
    iota_bc = np.tile(np.arange(P, dtype=np.float32)[None, :], (P, 1))
    ident = np.eye(P, dtype=np.float32)
    ones_row = np.ones((1, P), np.float32)
    in_maps = []
    for k in range(W):
        m = dict(
            node_feats=np.ascontiguousarray(inputs["node_feats"], np.float32),
            nf_loc=np.ascontiguousarray(pl.nf_loc[k]),
            ef_pad=np.ascontiguousarray(pl.ef_pad[k]),
            srcg=np.ascontiguousarray(pl.srcg[k]),
            hexti=np.ascontiguousarray(pl.hexti[k]),
            dstmod=np.ascontiguousarray(pl.dstmod[k]),
            rankcol=np.ascontiguousarray(pl.rankcol[k]),
            iota_bc=iota_bc, ident=ident, ones_row=ones_row,
            ones_col=np.ones((P, 1), np.float32),
        )
        for name, arr in wp.items():
            if name in ("scalars", "t_w", "r_proj_w"):
                continue
            m["w_" + name] = arr
        in_maps.append(m)
    return in_maps


def run(inputs, debug=False, trace=False, B=1000):
    node_feats = np.asarray(inputs["node_feats"], np.float32)
    edge_feats = np.asarray(inputs["edge_feats"], np.float32)
    src = np.asarray(inputs["src"], np.int64)
    dst = np.asarray(inputs["dst"], np.int64)
    graph_ids = np.asarray(inputs["graph_ids"], np.int64)
    G = inputs["et_w"].shape[0]
    PD = inputs["t_w"].shape[1]

    import time as _time
    _t = _time.time()
    pl = preprocess(node_feats, edge_feats, src, dst, graph_ids, B)
    wp = pack_weights(inputs, pl.F, pl.E, G, PD)
    print(f"[kernel] preprocess {_time.time()-_t:.1f}s NBLK={pl.NBLK} "
          f"TT={pl.TT}", flush=True)
    _t = _time.time()
    nc = build_program(pl, wp, debug=debug)
    print(f"[kernel] build {_time.time()-_t:.1f}s", flush=True)
    in_maps = make_in_maps(pl, inputs, wp)
    _t = _time.time()
    res = bass_utils.run_bass_kernel_spmd(
        nc, in_maps, core_ids=list(range(W)), trace=trace)
    print(f"[kernel] compile+run {_time.time()-_t:.1f}s", flush=True)

    out = np.zeros((B, PD), np.float32)
    # padded-rank row (empty-graph dynamics) — same on every core
    empty_row = res.results[0]["out"][P - 1]
    out[:] = empty_row
    for k in range(W):
        ch = pl.chunks[k]
        if len(ch):
            out[ch] = res.results[k]["out"][0:len(ch)]
    return out, res, pl


def kernel(**inputs):
    out, _, _ = run(inputs, debug=False, trace=False, B=1000)
    # match reference output dtype (float32)
    return out


# revision 16
# speedup vs baseline: 1.5160x; 1.5160x over previous
"""AttentiveFP (2-layer GNN + GRU readout) as a Bass/Tile kernel on 8 TRN2 cores.

Strategy (data-parallel over the graph batch):
  - Graphs (contiguous node segments, graph_ids sorted) are split into 8
    chunks balanced by node count; each core owns its chunk's nodes.
  - Edges are assigned to the core owning their dst node, sorted by dst, and
    padded per 128-node destination block to multiples of 128 (dummy edges
    carry an out-of-range dst sentinel so indicator matrices zero them out).
  - Segment softmax+sum over dst is computed per 128-edge tile as
    U = (e*Ind)^T @ [1|he1] on the TensorEngine, where Ind[e,j] = (dst_e==j)
    is built with a vector-engine is_equal against a constant iota matrix.
    1/sum normalization, linear layers + GRUCell run per 128-node block.
  - Between the two GNN layers, each core's [p2b|h] rows are AllGathered so
    layer 2 can gather h[src] rows (src is global) by indirect DMA.
  - Readout: each core owns <=125 non-empty graphs (rank-local ids); the
    same indicator machinery reduces nodes->graphs; 2 GRU timesteps; final
    linear produces [128, 128] per core; host scatters rows to [1000, 128].
"""
import sys

if "/opt/trn_rl_repo" not in sys.path:
    sys.path.insert(0, "/opt/trn_rl_repo")

import numpy as np
import concourse.bass as bass
import concourse.tile as tile
from concourse import mybir, bacc, bass_utils

P = 128
W = 8
F32 = mybir.dt.float32
BF16 = mybir.dt.bfloat16
I32 = mybir.dt.int32
NPBF16 = mybir.dt.np(mybir.dt.bfloat16)
AOT = mybir.AluOpType
ACTF = mybir.ActivationFunctionType
SENT = 999.0  # dst-sentinel for dummy edges / padded nodes
EPS = 1e-20
OCC_THRESH = 1e-10


# ----------------------------------------------------------------------------
# host-side preprocessing
# ----------------------------------------------------------------------------

class Plan:
    pass


def preprocess(node_feats, edge_feats, src, dst, graph_ids, B):
    N, F = node_feats.shape
    M, E = edge_feats.shape
    pl = Plan()
    pl.N, pl.F, pl.M, pl.E, pl.B = N, F, M, E, B

    gids = np.asarray(graph_ids)
    counts = np.bincount(gids, minlength=B)
    ne_ids = np.nonzero(counts)[0]
    chunks = np.array_split(ne_ids, W)
    # node range per core
    node_starts, node_counts = [], []
    for ch in chunks:
        if len(ch) == 0:
            node_starts.append(N)
            node_counts.append(0)
            continue
        s = int(np.searchsorted(gids, ch[0], "left"))
        e = int(np.searchsorted(gids, ch[-1], "right"))
        node_starts.append(s)
        node_counts.append(e - s)
    pl.chunks = chunks
    node_starts = np.array(node_starts, np.int64)
    node_counts = np.array(node_counts, np.int64)
    pl.node_starts = node_starts
    pl.node_counts = node_counts

    n_max = max(1, int(max(node_counts)))
    NLOC = -(-n_max // P) * P
    NBLK = NLOC // P
    pl.NLOC, pl.NBLK = NLOC, NBLK

    # per-core rank (graph index within core) for each local node
    rankcol = np.full((W, P, NBLK), SENT, np.float32)
    for k in range(W):
        ch = chunks[k]
        nk = node_counts[k]
        if nk == 0:
            continue
        g_local = gids[node_starts[k]:node_starts[k] + nk]
        # map graph id -> rank within chunk
        rk = np.searchsorted(ch, g_local)
        r = np.full(NLOC, SENT, np.float32)
        r[:nk] = rk.astype(np.float32)
        rankcol[k] = r.reshape(NBLK, P).T  # [P, NBLK]
    pl.rankcol = rankcol

    # local node feats
    nf_loc = np.zeros((W, NLOC, F), np.float32)
    for k in range(W):
        nk = node_counts[k]
        nf_loc[k, :nk] = node_feats[node_starts[k]:node_starts[k] + nk]
    pl.nf_loc = nf_loc

    # edges by dst owner
    owner = np.searchsorted(node_starts, dst, "right") - 1
    per_core = []
    blk_counts = np.zeros((W, NBLK), np.int64)
    for k in range(W):
        sel = np.nonzero(owner == k)[0]
        dloc = dst[sel] - node_starts[k]
        order = np.argsort(dloc, kind="stable")
        sel = sel[order]
        dloc = dloc[order]
        per_core.append((sel, dloc))
        bc = np.bincount(dloc // P, minlength=NBLK)
        blk_counts[k] = bc[:NBLK]
    TB = np.maximum(1, -(-blk_counts.max(0) // P)).astype(np.int64)  # tiles/blk
    pl.TB = TB
    TT = int(TB.sum())
    pl.TT = TT

    srcg = np.zeros((W, TT, P), np.int32)
    hexti = np.zeros((W, TT, P), np.int32)
    dstmod = np.full((W, TT, P), SENT, np.float32)
    ef_pad = np.zeros((W, TT, P, E), np.float32)
    tile_off = np.concatenate([[0], np.cumsum(TB)])[:-1]  # block -> first tile
    pl.tile_off = tile_off
    src_owner = np.searchsorted(node_starts, src, "right") - 1
    hext_row_of_src = (src_owner * NLOC + (src - node_starts[src_owner])).astype(
        np.int32)
    for k in range(W):
        sel, dloc = per_core[k]
        blk = dloc // P
        # position within block
        for b in range(NBLK):
            m = blk == b
            cnt = int(m.sum())
            if cnt == 0:
                continue
            es = sel[m]
            t0 = tile_off[b]
            flat = np.arange(cnt)
            t_idx = t0 + flat // P
            p_idx = flat % P
            srcg[k, t_idx, p_idx] = src[es]
            hexti[k, t_idx, p_idx] = hext_row_of_src[es]
            dstmod[k, t_idx, p_idx] = (dloc[m] % P).astype(np.float32)
            ef_pad[k, t_idx, p_idx] = edge_feats[es]
    pl.srcg, pl.hexti, pl.dstmod, pl.ef_pad = srcg, hexti, dstmod, ef_pad
    return pl


# ----------------------------------------------------------------------------
# kernel builder
# ----------------------------------------------------------------------------

class Ctx:
    pass


def _transpose(cx, src_ap, k, dtype=None):
    """PE-transpose src_ap [P, k] -> SBUF [k, P]."""
    nc = cx.nc
    if dtype is None:
        dtype = cx.cdt
    tps = cx.psum.tile([P, P], F32, space="PSUM", tag="tr", bufs=2,
                       name="tps")
    nc.tensor.transpose(out=tps[:k, :P], in_=src_ap, identity=cx.ident[:, :])
    tsb = cx.work.tile([P, P], dtype, tag="tsb", bufs=4, name="tsb")
    nc.vector.tensor_copy(tsb[:k, :P], tps[:k, :P])
    return tsb[:k, :P]


def _bcast_col(cx, col_ap):
    """[P,1] column -> [P,P] matrix whose every row is col^T."""
    nc = cx.nc
    tps = cx.psum.tile([P, P], F32, space="PSUM", tag="tr", bufs=2, name="tps")
    nc.tensor.transpose(out=tps[:1, :P], in_=col_ap, identity=cx.ident[:, :])
    prow = cx.work.tile([1, P], F32, tag="prow", bufs=2, name="prow")
    nc.vector.tensor_copy(prow[:, :], tps[:1, :P])
    bps = cx.psum.tile([P, P], F32, space="PSUM", tag="gps", bufs=3, name="bps")
    nc.tensor.matmul(out=bps[:, :], lhsT=cx.ones_row[:, :], rhs=prow[:, :],
                     start=True, stop=True)
    bsb = cx.work.tile([P, P], F32, tag="bsb", bufs=2, name="bsb")
    nc.vector.tensor_copy(bsb[:, :], bps[:, :])
    return bsb


def _leaky(cx, out_ap, in_ap, tmp_tag="lk"):
    nc = cx.nc
    if cx.act_leaky:
        nc.scalar.activation(out_ap, in_ap, ACTF.Lrelu, alpha=0.01)
        return
    shape = [in_ap.shape[0], in_ap.shape[1]]
    tmp = cx.work.tile([P, 256], F32, tag=tmp_tag, bufs=2, name="lktmp")
    t = tmp[:shape[0], :shape[1]]
    nc.vector.tensor_scalar_mul(t, in_ap, 0.01)
    nc.vector.tensor_tensor(out=out_ap, in0=in_ap, in1=t, op=AOT.max)


def _elu(cx, out_ap, in_ap):
    """out = elu(in); in may be PSUM."""
    nc = cx.nc
    n, m = in_ap.shape[0], in_ap.shape[1]
    mn = cx.work.tile([P, 256], F32, tag="elu1", bufs=2, name="elmn")[:n, :m]
    ex = cx.work.tile([P, 256], F32, tag="elu2", bufs=2, name="elex")[:n, :m]
    rl = cx.work.tile([P, 256], F32, tag="elu3", bufs=2, name="elrl")[:n, :m]
    nc.vector.tensor_scalar_min(mn, in_ap, 0.0)
    nc.scalar.activation(ex, mn, ACTF.Exp)
    nc.vector.tensor_scalar_max(rl, in_ap, 0.0)
    nc.vector.tensor_scalar_add(ex, ex, -1.0)
    nc.vector.tensor_tensor(out=out_ap, in0=ex, in1=rl, op=AOT.add)


def _sigmoid(cx, out_ap, in_ap):
    """out = sigmoid(in) = 0.5*tanh(0.5x)+0.5; in may be PSUM."""
    nc = cx.nc
    nc.scalar.activation(out_ap, in_ap, ACTF.Tanh, scale=0.5)
    nc.vector.tensor_scalar(out=out_ap, in0=out_ap, scalar1=1.0, scalar2=0.5,
                            op0=AOT.add, op1=AOT.mult)


def _gru(cx, x_ap, h_ap, wname, out_ap, relu):
    """GRUCell for one 128-row block. x,h: [P,200] SBUF. out_ap: [P,200]."""
    nc = cx.nc
    G = cx.G
    wih0, wih1 = cx.consts[wname + "_wih0"], cx.consts[wname + "_wih1"]
    whh0, whh1 = cx.consts[wname + "_whh0"], cx.consts[wname + "_whh1"]
    brz = cx.consts[wname + "_brz"]
    bihn = cx.consts[wname + "_bihn"]
    bhhn = cx.consts[wname + "_bhhn"]
    H = G // 2
    xt0 = _transpose(cx, x_ap[:, 0:H], H)
    xt1 = _transpose(cx, x_ap[:, H:G], H)
    ht0 = _transpose(cx, h_ap[:, 0:H], H)
    ht1 = _transpose(cx, h_ap[:, H:G], H)

    a = cx.psum.tile([P, 2 * G], F32, space="PSUM", tag="gps", bufs=3,
                     name="gruA")
    nc.tensor.matmul(out=a[:, :], lhsT=xt0, rhs=wih0[:, 0:2 * G], start=True,
                     stop=False)
    nc.tensor.matmul(out=a[:, :], lhsT=xt1, rhs=wih1[:, 0:2 * G], start=False,
                     stop=False)
    nc.tensor.matmul(out=a[:, :], lhsT=ht0, rhs=whh0[:, 0:2 * G], start=False,
                     stop=False)
    nc.tensor.matmul(out=a[:, :], lhsT=ht1, rhs=whh1[:, 0:2 * G], start=False,
                     stop=False)
    nc.tensor.matmul(out=a[:, :], lhsT=cx.ones_row_c[:, :], rhs=brz[:, :],
                     start=False, stop=True)
    xn = cx.psum.tile([P, 2 * G], F32, space="PSUM", tag="gps", bufs=3,
                      name="gruXN")
    nc.tensor.matmul(out=xn[:, 0:G], lhsT=xt0, rhs=wih0[:, 2 * G:3 * G],
                     start=True, stop=False)
    nc.tensor.matmul(out=xn[:, 0:G], lhsT=xt1, rhs=wih1[:, 2 * G:3 * G],
                     start=False, stop=False)
    nc.tensor.matmul(out=xn[:, 0:G], lhsT=cx.ones_row_c[:, :],
                     rhs=bihn[:, :], start=False, stop=True)
    hn = cx.psum.tile([P, 2 * G], F32, space="PSUM", tag="gps", bufs=3,
                      name="gruHN")
    nc.tensor.matmul(out=hn[:, 0:G], lhsT=ht0, rhs=whh0[:, 2 * G:3 * G],
                     start=True, stop=False)
    nc.tensor.matmul(out=hn[:, 0:G], lhsT=ht1, rhs=whh1[:, 2 * G:3 * G],
                     start=False, stop=False)
    nc.tensor.matmul(out=hn[:, 0:G], lhsT=cx.ones_row_c[:, :],
                     rhs=bhhn[:, :], start=False, stop=True)

    rz = cx.work.tile([P, 2 * G], F32, tag="rz", bufs=2, name="rz")
    _sigmoid(cx, rz[:, :], a[:, :])
    t1 = cx.work.tile([P, G], F32, tag="gt1", bufs=2, name="gt1")
    nc.vector.tensor_tensor(out=t1[:, :], in0=rz[:, 0:G], in1=hn[:, 0:G],
                            op=AOT.mult)
    t2 = cx.work.tile([P, G], F32, tag="gt2", bufs=2, name="gt2")
    nc.vector.tensor_tensor(out=t2[:, :], in0=t1[:, :], in1=xn[:, 0:G],
                            op=AOT.add)
    n_ = cx.work.tile([P, G], F32, tag="gn", bufs=2, name="gn")
    nc.scalar.activation(n_[:, :], t2[:, :], ACTF.Tanh)
    d = cx.work.tile([P, G], F32, tag="gd", bufs=2, name="gd")
    nc.vector.tensor_tensor(out=d[:, :], in0=h_ap, in1=n_[:, :], op=AOT.subtract)
    e2 = cx.work.tile([P, G], F32, tag="ge", bufs=2, name="ge")
    nc.vector.tensor_tensor(out=e2[:, :], in0=rz[:, G:2 * G], in1=d[:, :],
                            op=AOT.mult)
    if relu:
        hn2 = cx.work.tile([P, G], F32, tag="gh", bufs=2, name="gh")
        nc.vector.tensor_tensor(out=hn2[:, :], in0=n_[:, :], in1=e2[:, :],
                                op=AOT.add)
        nc.vector.tensor_scalar_max(out_ap, hn2[:, :], 0.0)
    else:
        nc.vector.tensor_tensor(out=out_ap, in0=n_[:, :], in1=e2[:, :],
                                op=AOT.add)


def _ttr(cx, in0, in1, scalar, accum_out, width):
    """accum_out[p] = sum_f(in0*in1) + scalar.  (tensor_tensor_reduce is
    broken on HW, so this is mult + reduce + add.)"""
    nc = cx.nc
    scr = cx.work.tile([P, 256], F32, tag="ttrscr", bufs=2, name="ttrscr")
    s = scr[:, 0:width]
    nc.vector.tensor_tensor(out=s, in0=in0, in1=in1, op=AOT.mult)
    nc.vector.reduce_sum(accum_out, s, axis=mybir.AxisListType.X)
    if isinstance(scalar, float):
        if scalar != 0.0:
            nc.vector.tensor_scalar_add(accum_out, accum_out, scalar)
    else:
        nc.vector.tensor_tensor(out=accum_out, in0=accum_out, in1=scalar,
                                op=AOT.add)


def build_program(pl, weights, debug=False, bf16=True, act_leaky=True):
    """Build the full Bass program. weights: dict of numpy arrays."""
    G = 200
    PD = weights["t_w"].shape[1]
    NBLK, TT, TB = pl.NBLK, pl.TT, pl.TB
    NLOC = pl.NLOC
    E, F = pl.E, pl.F
    T_steps = weights["r_proj_w"].shape[0]

    nc = bacc.Bacc("TRN2", num_devices=W, debug=False)
    cx = Ctx()
    cx.nc = nc
    cx.G = G
    cx.cdt = BF16 if bf16 else F32
    cx.act_leaky = act_leaky
    CDT = cx.cdt

    # ---- external inputs -------------------------------------------------
    def inp(name, shape, dt=F32):
        return nc.dram_tensor(name, list(shape), dt, kind="ExternalInput")

    t_nf_full = inp("node_feats", [pl.N, F])
    t_nf_loc = inp("nf_loc", [NLOC, F])
    t_ef = inp("ef_pad", [TT, P, E])
    t_srcg = inp("srcg", [TT, P], I32)
    t_hexti = inp("hexti", [TT, P], I32)
    t_dstmod = inp("dstmod", [TT, P])
    t_rankcol = inp("rankcol", [P, NBLK])
    t_iota = inp("iota_bc", [P, P])
    t_ident = inp("ident", [P, P])
    t_ones_row = inp("ones_row", [1, P])
    t_ones_col = inp("ones_col", [P, 1])
    w_in = {}
    wspec = {
        "pnw_ext": [F + 1, G], "pe1w_ext": [F + E + 1, G],
        "w2a_bc": [P, G], "w2b_bc": [P, G],
        "w2a2_bc": [P, G], "w2b2_bc": [P, G],
        "etw": [G, G], "etb_row": [1, G],
        "lpnw": [G, G], "lpnb_row": [1, G],
        "tw": [G, PD], "tb_row": [1, PD],
    }
    for t in range(T_steps):
        wspec[f"wA{t}_bc"] = [P, G]
        wspec[f"wB{t}_bc"] = [P, G]
        wspec[f"rpw{t}"] = [G, G]
        wspec[f"rpb{t}_row"] = [1, G]
    for nm in ["g0", "g1"] + [f"gr{t}" for t in range(T_steps)]:
        wspec[nm + "_wih"] = [G, 3 * G]
        wspec[nm + "_whh"] = [G, 3 * G]
        wspec[nm + "_brz"] = [1, 2 * G]
        wspec[nm + "_bihn"] = [1, G]
        wspec[nm + "_bhhn"] = [1, G]
    for name, shp in wspec.items():
        w_in[name] = inp("w_" + name, shp,
                         F32 if name.endswith("_bc") else CDT)

    t_out = nc.dram_tensor("out", [P, PD], F32, kind="ExternalOutput")
    dbg = {}
    if debug:
        for nm, shp in [("hv_dbg", [NLOC, G]), ("h1_dbg", [NLOC, G]),
                        ("h2_dbg", [NLOC, G]), ("g0_dbg", [P, G + 1]),
                        ("u1_dbg", [NLOC, G + 1]), ("p1_dbg", [NLOC, 1]),
                        ("gf_dbg", [P, G])]:
            dbg[nm] = nc.dram_tensor(nm, shp, F32, kind="ExternalOutput")

    TMAX = int(TB.max())

    with tile.TileContext(nc) as tc:
        with tc.tile_pool(name="const", bufs=1) as cpool, \
             tc.tile_pool(name="work", bufs=2) as work, \
             tc.tile_pool(name="psum", bufs=2, space="PSUM") as psum, \
             tc.tile_pool(name="dram", bufs=1, space="DRAM") as dram:
            cx.work, cx.psum = work, psum

            # ---- persistent consts --------------------------------------
            cx.consts = {}

            def load_const(name, src_ap, shape, dt=F32):
                t = cpool.tile(shape, dt, tag="c_" + name, name="c_" + name)
                nc.sync.dma_start(out=t[:, :], in_=src_ap)
                cx.consts[name] = t
                return t

            cx.ident = load_const("ident", t_ident[:, :], [P, P])
            cx.iota = load_const("iota", t_iota[:, :], [P, P])
            cx.ones_row = load_const("ones_row", t_ones_row[:, :], [1, P])
            cx.ones_col = load_const("ones_col", t_ones_col[:, :], [P, 1])
            cx.ones_row_c = cpool.tile([1, P], CDT, tag="c_ones_row_c",
                                       name="c_ones_row_c")
            nc.vector.tensor_copy(cx.ones_row_c[:, :], cx.ones_row[:, :])
            load_const("pe1w_nf", w_in["pe1w_ext"][0:F, :], [F, G], CDT)
            load_const("pe1w_ef", w_in["pe1w_ext"][F:F + E, :], [E, G], CDT)
            load_const("pe1w_b", w_in["pe1w_ext"][F + E:F + E + 1, :], [1, G],
                       CDT)
            for name, shp in wspec.items():
                if name == "pe1w_ext":
                    continue
                if name.endswith("_wih") or name.endswith("_whh") or                         name in ("etw", "lpnw", "tw") or name.startswith("rpw"):
                    base = w_in[name]
                    load_const(name + "0", base[0:G // 2, :], [G // 2, shp[1]],
                               CDT)
                    load_const(name + "1", base[G // 2:G, :], [G // 2, shp[1]],
                               CDT)
                else:
                    load_const(name, w_in[name][:, :], shp,
                               F32 if name.endswith("_bc") else CDT)

            rankcol = load_const("rankcol", t_rankcol[:, :], [P, NBLK])
            p2a_all = cpool.tile([P, NBLK], F32, tag="p2a_all", name="p2a_all")

            hext_loc = dram.tile([NLOC, G + 2], CDT, name="hext_loc")
            hext_glob = dram.tile([W * NLOC, G + 2], CDT, name="hext_glob")

            hblk = [cpool.tile([P, G + 2], F32, tag=f"hblk{b}",
                               name=f"hblk{b}") for b in range(NBLK)]

            scalars = weights["scalars"]
            pe2_b = float(scalars["pe2_b"])
            lpe_b = float(scalars["lpe_b"])
            rlb = [float(x) for x in scalars["r_logit_b"]]

            # =============================================================
            # Layer 1 + GRU0, block by block
            # =============================================================
            for b in range(NBLK):
                tb = int(TB[b])
                t0 = int(pl.tile_off[b])
                # --- hv_new for this block -------------------------------
                nfx = work.tile([P, F + 1], F32, tag="nfx", bufs=2, name="nfx")
                nc.sync.dma_start(out=nfx[:, 0:F],
                                  in_=t_nf_loc[b * P:(b + 1) * P, :])
                nc.gpsimd.memset(nfx[:, F:F + 1], 1.0)
                nfxt = _transpose(cx, nfx[:, :], F + 1)
                hvps = psum.tile([P, G], F32, space="PSUM", tag="he1ps",
                                 bufs=1, name="hvps")
                nc.tensor.matmul(out=hvps[:, :], lhsT=nfxt,
                                 rhs=cx.consts["pnw_ext"][:, :], start=True,
                                 stop=True)
                hv = work.tile([P, G], F32, tag="hv", bufs=3, name="hv")
                _leaky(cx, hv[:, :], hvps[:, :])
                # p1 = hv @ pe2_w[:200] + pe2_b
                p1 = work.tile([P, 1], F32, tag="p1", bufs=2, name="p1")
                _ttr(cx, hv[:, :], cx.consts["w2a_bc"][:, :], pe2_b,
                     p1[:, 0:1], G)
                p_bc = _bcast_col(cx, p1[:, 0:1])
                if debug:
                    nc.sync.dma_start(out=dbg["hv_dbg"][b * P:(b + 1) * P, :],
                                      in_=hv[:, :])
                    nc.sync.dma_start(out=dbg["p1_dbg"][b * P:(b + 1) * P, :],
                                      in_=p1[:, :])

                # --- edge tiles: stage A ---------------------------------
                zst = work.tile([P, TMAX], F32, tag="zst", bufs=2, name="zst")
                efb = work.tile([P, TMAX * E], F32, tag="efb", bufs=2,
                                name="efb")
                efb3 = efb.rearrange("p (t e) -> p t e", t=TMAX)
                nc.sync.dma_start(
                    out=efb3[:, 0:tb, :],
                    in_=t_ef[t0:t0 + tb, :, :].rearrange("t p e -> p t e"))
                dmb = work.tile([P, TMAX], F32, tag="dmb", bufs=2, name="dmb")
                nc.sync.dma_start(
                    out=dmb[:, 0:tb],
                    in_=t_dstmod[t0:t0 + tb, :].rearrange("t p -> p t"))
                inds = []
                he1s = []
                for t in range(tb):
                    gt = t0 + t
                    xg = work.tile([P, F], F32, tag="xg", bufs=4, name="xg")
                    idxt = work.tile([P, 1], I32, tag="idxt", bufs=4,
                                     name="idxt")
                    nc.sync.dma_start(
                        out=idxt[:, :],
                        in_=t_srcg[gt:gt + 1, :].rearrange("a p -> p a"))
                    nc.gpsimd.indirect_dma_start(
                        out=xg[:, :], out_offset=None, in_=t_nf_full[:, :],
                        in_offset=bass.IndirectOffsetOnAxis(ap=idxt[:, :],
                                                            axis=0))
                    xgt = _transpose(cx, xg[:, :], F)
                    eftt = _transpose(cx, efb3[:, t, :], E)
                    h1ps = psum.tile([P, G], F32, space="PSUM", tag="he1ps",
                                     bufs=1, name="h1ps")
                    nc.tensor.matmul(out=h1ps[:, :], lhsT=xgt,
                                     rhs=cx.consts["pe1w_nf"][:, :],
                                     start=True, stop=False)
                    nc.tensor.matmul(out=h1ps[:, :], lhsT=eftt,
                                     rhs=cx.consts["pe1w_ef"][:, :],
                                     start=False, stop=False)
                    nc.tensor.matmul(out=h1ps[:, :], lhsT=cx.ones_row_c[:, :],
                                     rhs=cx.consts["pe1w_b"][:, :],
                                     start=False, stop=True)
                    he1 = work.tile([P, G + 4], CDT, tag="he1",
                                    bufs=TMAX + 1, name="he1")
                    nc.gpsimd.memset(he1[:, 0:1], 1.0)
                    _leaky(cx, he1[:, 1:G + 1], h1ps[:, :])
                    ind = work.tile([P, P], CDT, tag="ind", bufs=TMAX + 1,
                                    name="ind")
                    nc.vector.tensor_scalar(
                        out=ind[:, :], in0=cx.iota[:, :],
                        scalar1=dmb[:, t:t + 1],
                        scalar2=None, op0=AOT.is_equal)
                    za = work.tile([P, 1], F32, tag="za", bufs=4, name="za")
                    _ttr(cx, he1[:, 1:G + 1], cx.consts["w2b_bc"][:, :], 0.0,
                         za[:, 0:1], G)
                    _ttr(cx, ind[:, :], p_bc[:, :], za[:, 0:1],
                         zst[:, t:t + 1], P)
                    inds.append(ind)
                    he1s.append(he1)
                # --- exp over the block ----------------------------------
                zl = work.tile([P, TMAX], F32, tag="zl", bufs=2, name="zl")
                _leaky(cx, zl[:, 0:tb], zst[:, 0:tb], tmp_tag="lkz")
                ee = work.tile([P, TMAX], F32, tag="ee", bufs=2, name="ee")
                nc.scalar.activation(ee[:, 0:tb], zl[:, 0:tb], ACTF.Exp)
                # --- stage B: weighted segment reduction -----------------
                u1 = psum.tile([P, G + 1], F32, space="PSUM", tag="U",
                               bufs=2, name="u1")
                for t in range(tb):
                    wt = work.tile([P, P], CDT, tag="wt", bufs=3, name="wt")
                    nc.vector.tensor_scalar_mul(wt[:, :], inds[t][:, :],
                                                ee[:, t:t + 1])
                    nc.tensor.matmul(out=u1[:, :], lhsT=wt[:, :],
                                     rhs=he1s[t][:, 0:G + 1],
                                     start=(t == 0), stop=(t == tb - 1))
                if debug:
                    u1sb = work.tile([P, G + 1], F32, tag="u1sb", bufs=2,
                                     name="u1sb")
                    nc.vector.tensor_copy(u1sb[:, :], u1[:, :])
                    nc.sync.dma_start(out=dbg["u1_dbg"][b * P:(b + 1) * P, :],
                                      in_=u1sb[:, :])
                # --- normalize + linear + GRU0 ---------------------------
                s_sb = work.tile([P, 1], F32, tag="ssb", bufs=2, name="ssb")
                nc.vector.tensor_scalar_add(s_sb[:, :], u1[:, 0:1], EPS)
                rec = work.tile([P, 1], F32, tag="rec", bufs=2, name="rec")
                nc.vector.reciprocal(rec[:, :], s_sb[:, :])
                occ = work.tile([P, 1], F32, tag="occ", bufs=2, name="occ")
                nc.vector.tensor_scalar(out=occ[:, :], in0=s_sb[:, :],
                                        scalar1=OCC_THRESH, scalar2=None,
                                        op0=AOT.is_ge)
                csum = work.tile([P, G], F32, tag="csum", bufs=2, name="csum")
                nc.vector.tensor_scalar_mul(csum[:, :], u1[:, 1:G + 1],
                                            rec[:, 0:1])
                ct0 = _transpose(cx, csum[:, 0:G // 2], G // 2)
                ct1 = _transpose(cx, csum[:, G // 2:G], G // 2)
                occ_t = _transpose(cx, occ[:, 0:1], 1)
                cg = psum.tile([P, G], F32, space="PSUM", tag="gps", bufs=3,
                               name="cg")
                nc.tensor.matmul(out=cg[:, :], lhsT=ct0,
                                 rhs=cx.consts["etw0"][:, :], start=True,
                                 stop=False)
                nc.tensor.matmul(out=cg[:, :], lhsT=ct1,
                                 rhs=cx.consts["etw1"][:, :], start=False,
                                 stop=False)
                nc.tensor.matmul(out=cg[:, :], lhsT=occ_t,
                                 rhs=cx.consts["etb_row"][:, :], start=False,
                                 stop=True)
                x1 = work.tile([P, G], F32, tag="x1", bufs=2, name="x1")
                _elu(cx, x1[:, :], cg[:, :])
                _gru(cx, x1[:, :], hv[:, :], "g0", hblk[b][:, 1:G + 1],
                     relu=True)
                # p2a / p2b
                _ttr(cx, hblk[b][:, 1:G + 1], cx.consts["w2a2_bc"][:, :],
                     lpe_b, p2a_all[:, b:b + 1], G)
                _ttr(cx, hblk[b][:, 1:G + 1], cx.consts["w2b2_bc"][:, :], 0.0,
                     hblk[b][:, 0:1], G)
                nc.gpsimd.memset(hblk[b][:, G + 1:G + 2], 1.0)
                nc.gpsimd.dma_start(out=hext_loc[b * P:(b + 1) * P, :],
                                    in_=hblk[b][:, 0:G + 2])
                if debug:
                    nc.sync.dma_start(out=dbg["h1_dbg"][b * P:(b + 1) * P, :],
                                      in_=hblk[b][:, 1:G + 1])

            # =============================================================
            # AllGather h_ext
            # =============================================================
            nc.gpsimd.collective_compute(
                "AllGather", AOT.bypass, replica_groups=[list(range(W))],
                ins=[hext_loc.opt()], outs=[hext_glob.opt()])

            # =============================================================
            # Layer 2 + GRU1, block by block
            # =============================================================
            for b in range(NBLK):
                tb = int(TB[b])
                t0 = int(pl.tile_off[b])
                p2_bc = _bcast_col(cx, p2a_all[:, b:b + 1])
                zst = work.tile([P, TMAX], F32, tag="zst", bufs=2, name="zst")
                dmb = work.tile([P, TMAX], F32, tag="dmb", bufs=2, name="dmb")
                nc.sync.dma_start(
                    out=dmb[:, 0:tb],
                    in_=t_dstmod[t0:t0 + tb, :].rearrange("t p -> p t"))
                inds = []
                etgs = []
                for t in range(tb):
                    gt = t0 + t
                    etg = work.tile([P, G + 2], CDT, tag="etg",
                                    bufs=TMAX + 1, name="etg")
                    idxt = work.tile([P, 1], I32, tag="idxt", bufs=4,
                                     name="idxt")
                    nc.sync.dma_start(
                        out=idxt[:, :],
                        in_=t_hexti[gt:gt + 1, :].rearrange("a p -> p a"))
                    nc.gpsimd.indirect_dma_start(
                        out=etg[:, :], out_offset=None,
                        in_=hext_glob[:, :],
                        in_offset=bass.IndirectOffsetOnAxis(ap=idxt[:, :],
                                                            axis=0))
                    ind = work.tile([P, P], CDT, tag="ind", bufs=TMAX + 1,
                                    name="ind")
                    nc.vector.tensor_scalar(
                        out=ind[:, :], in0=cx.iota[:, :],
                        scalar1=dmb[:, t:t + 1],
                        scalar2=None, op0=AOT.is_equal)
                    _ttr(cx, ind[:, :], p2_bc[:, :], etg[:, 0:1],
                         zst[:, t:t + 1], P)
                    inds.append(ind)
                    etgs.append(etg)
                zl = work.tile([P, TMAX], F32, tag="zl", bufs=2, name="zl")
                _leaky(cx, zl[:, 0:tb], zst[:, 0:tb], tmp_tag="lkz")
                ee = work.tile([P, TMAX], F32, tag="ee", bufs=2, name="ee")
                nc.scalar.activation(ee[:, 0:tb], zl[:, 0:tb], ACTF.Exp)
                u2 = psum.tile([P, G + 1], F32, space="PSUM", tag="U",
                               bufs=2, name="u2")
                for t in range(tb):
                    wt = work.tile([P, P], CDT, tag="wt", bufs=3, name="wt")
                    nc.vector.tensor_scalar_mul(wt[:, :], inds[t][:, :],
                                                ee[:, t:t + 1])
                    nc.tensor.matmul(out=u2[:, :], lhsT=wt[:, :],
                                     rhs=etgs[t][:, 1:G + 2],
                                     start=(t == 0), stop=(t == tb - 1))
                s_sb = work.tile([P, 1], F32, tag="ssb", bufs=2, name="ssb")
                nc.vector.tensor_scalar_add(s_sb[:, :], u2[:, G:G + 1], EPS)
                rec = work.tile([P, 1], F32, tag="rec", bufs=2, name="rec")
                nc.vector.reciprocal(rec[:, :], s_sb[:, :])
                occ = work.tile([P, 1], F32, tag="occ", bufs=2, name="occ")
                nc.vector.tensor_scalar(out=occ[:, :], in0=s_sb[:, :],
                                        scalar1=OCC_THRESH, scalar2=None,
                                        op0=AOT.is_ge)
                csum = work.tile([P, G], F32, tag="csum", bufs=2, name="csum")
                nc.vector.tensor_scalar_mul(csum[:, :], u2[:, 0:G],
                                            rec[:, 0:1])
                ct0 = _transpose(cx, csum[:, 0:G // 2], G // 2)
                ct1 = _transpose(cx, csum[:, G // 2:G], G // 2)
                occ_t = _transpose(cx, occ[:, 0:1], 1)
                cg = psum.tile([P, G], F32, space="PSUM", tag="gps", bufs=3,
                               name="cg2")
                nc.tensor.matmul(out=cg[:, :], lhsT=ct0,
                                 rhs=cx.consts["lpnw0"][:, :], start=True,
                                 stop=False)
                nc.tensor.matmul(out=cg[:, :], lhsT=ct1,
                                 rhs=cx.consts["lpnw1"][:, :], start=False,
                                 stop=False)
                nc.tensor.matmul(out=cg[:, :], lhsT=occ_t,
                                 rhs=cx.consts["lpnb_row"][:, :], start=False,
                                 stop=True)
                x2 = work.tile([P, G], F32, tag="x1", bufs=2, name="x2")
                _elu(cx, x2[:, :], cg[:, :])
                _gru(cx, x2[:, :], hblk[b][:, 1:G + 1], "g1",
                     hblk[b][:, 1:G + 1], relu=True)
                if debug:
                    nc.sync.dma_start(out=dbg["h2_dbg"][b * P:(b + 1) * P, :],
                                      in_=hblk[b][:, 1:G + 1])

            # =============================================================
            # Readout
            # =============================================================
            def mk_indg(b):
                t = work.tile([P, P], F32, tag="indg", bufs=3, name="indg")
                nc.vector.tensor_scalar(
                    out=t[:, :], in0=cx.iota[:, :],
                    scalar1=rankcol[:, b:b + 1], scalar2=None,
                    op0=AOT.is_equal)
                return t

            g0ps = psum.tile([P, G + 1], F32, space="PSUM", tag="U", bufs=2,
                             name="g0ps")
            for b in range(NBLK):
                gi = mk_indg(b)
                nc.tensor.matmul(out=g0ps[:, :], lhsT=gi[:, :],
                                 rhs=hblk[b][:, 1:G + 2], start=(b == 0),
                                 stop=(b == NBLK - 1))
            g_sb = work.tile([P, G], F32, tag="gsb", bufs=2, name="gsb")
            nc.vector.tensor_copy(g_sb[:, :], g0ps[:, 0:G])
            occg = work.tile([P, 1], F32, tag="occg", bufs=1, name="occg")
            nc.vector.tensor_scalar(out=occg[:, :], in0=g0ps[:, G:G + 1],
                                    scalar1=0.5, scalar2=None, op0=AOT.is_ge)
            occg_t = _transpose(cx, occg[:, 0:1], 1)
            occg_tc = cpool.tile([1, P], CDT, tag="occgt", name="occgt")
            nc.vector.tensor_copy(occg_tc[:, :], occg_t)
            if debug:
                g0sb = work.tile([P, G + 1], F32, tag="g0sb", bufs=1,
                                 name="g0sb")
                nc.vector.tensor_copy(g0sb[:, :], g0ps[:, :])
                nc.sync.dma_start(out=dbg["g0_dbg"][:, :], in_=g0sb[:, :])

            for ts in range(T_steps):
                grelu = work.tile([P, G], F32, tag="grelu", bufs=2,
                                  name="grelu")
                nc.vector.tensor_scalar_max(grelu[:, :], g_sb[:, :], 0.0)
                q1 = work.tile([P, 1], F32, tag="q1", bufs=2, name="q1")
                _ttr(cx, grelu[:, :], cx.consts[f"wA{ts}_bc"][:, :], rlb[ts],
                     q1[:, 0:1], G)
                q_bc = _bcast_col(cx, q1[:, 0:1])
                zstr = work.tile([P, NBLK], F32, tag="zstr", bufs=2,
                                 name="zstr")
                for b in range(NBLK):
                    za = work.tile([P, 1], F32, tag="za", bufs=4, name="za")
                    _ttr(cx, hblk[b][:, 1:G + 1], cx.consts[f"wB{ts}_bc"][:, :],
                         0.0, za[:, 0:1], G)
                    gi = mk_indg(b)
                    _ttr(cx, gi[:, :], q_bc[:, :], za[:, 0:1],
                         zstr[:, b:b + 1], P)
                zlr = work.tile([P, NBLK], F32, tag="zlr", bufs=2, name="zlr")
                _leaky(cx, zlr[:, :], zstr[:, :], tmp_tag="lkz")
                eer = work.tile([P, NBLK], F32, tag="eer", bufs=2, name="eer")
                nc.scalar.activation(eer[:, :], zlr[:, :], ACTF.Exp)
                ur = psum.tile([P, G + 1], F32, space="PSUM", tag="U", bufs=2,
                               name="ur")
                for b in range(NBLK):
                    gi = mk_indg(b)
                    wt = work.tile([P, P], F32, tag="wt", bufs=3, name="wt")
                    nc.vector.tensor_scalar_mul(wt[:, :], gi[:, :],
                                                eer[:, b:b + 1])
                    nc.tensor.matmul(out=ur[:, :], lhsT=wt[:, :],
                                     rhs=hblk[b][:, 1:G + 2], start=(b == 0),
                                     stop=(b == NBLK - 1))
                s_sb = work.tile([P, 1], F32, tag="ssb", bufs=2, name="ssb")
                nc.vector.tensor_scalar_add(s_sb[:, :], ur[:, G:G + 1], EPS)
                rec = work.tile([P, 1], F32, tag="rec", bufs=2, name="rec")
                nc.vector.reciprocal(rec[:, :], s_sb[:, :])
                gnum = work.tile([P, G], F32, tag="csum", bufs=2, name="gnum")
                nc.vector.tensor_scalar_mul(gnum[:, :], ur[:, 0:G],
                                            rec[:, 0:1])
                gt0 = _transpose(cx, gnum[:, 0:G // 2], G // 2)
                gt1 = _transpose(cx, gnum[:, G // 2:G], G // 2)
                gr = psum.tile([P, G], F32, space="PSUM", tag="gps", bufs=3,
                               name="gr")
                nc.tensor.matmul(out=gr[:, :], lhsT=gt0,
                                 rhs=cx.consts[f"rpw{ts}0"][:, :], start=True,
                                 stop=False)
                nc.tensor.matmul(out=gr[:, :], lhsT=gt1,
                                 rhs=cx.consts[f"rpw{ts}1"][:, :], start=False,
                                 stop=False)
                nc.tensor.matmul(out=gr[:, :], lhsT=occg_tc[:, :],
                                 rhs=cx.consts[f"rpb{ts}_row"][:, :],
                                 start=False, stop=True)
                xr = work.tile([P, G], F32, tag="x1", bufs=2, name="xr")
                _elu(cx, xr[:, :], gr[:, :])
                gnew = work.tile([P, G], F32, tag="gsb", bufs=2, name="gnew")
                _gru(cx, xr[:, :], g_sb[:, :], f"gr{ts}", gnew[:, :],
                     relu=False)
                g_sb = gnew

            if debug:
                nc.sync.dma_start(out=dbg["gf_dbg"][:, :], in_=g_sb[:, :])
            ft0 = _transpose(cx, g_sb[:, 0:G // 2], G // 2)
            ft1 = _transpose(cx, g_sb[:, G // 2:G], G // 2)
            ops = psum.tile([P, PD], F32, space="PSUM", tag="gps", bufs=3,
                            name="ops")
            nc.tensor.matmul(out=ops[:, :], lhsT=ft0,
                             rhs=cx.consts["tw0"][:, :], start=True,
                             stop=False)
            nc.tensor.matmul(out=ops[:, :], lhsT=ft1,
                             rhs=cx.consts["tw1"][:, :], start=False,
                             stop=False)
            nc.tensor.matmul(out=ops[:, :], lhsT=cx.ones_row_c[:, :],
                             rhs=cx.consts["tb_row"][:, :], start=False,
                             stop=True)
            osb = work.tile([P, PD], F32, tag="osb", bufs=1, name="osb")
            nc.vector.tensor_copy(osb[:, :], ops[:, :])
            nc.sync.dma_start(out=t_out[:, :], in_=osb[:, :])

    nc.compile()
    return nc


# ----------------------------------------------------------------------------
# weight packing (shared by all cores)
# ----------------------------------------------------------------------------

def pack_weights(i, F, E, G, PD):
    def bc(row):
        return np.tile(np.asarray(row, np.float32).reshape(1, -1), (P, 1))

    T_steps = i["r_proj_w"].shape[0]
    wp = {}
    wp["pnw_ext"] = np.concatenate([i["pn_w"], i["pn_b"][None, :]], 0)
    wp["pe1w_ext"] = np.concatenate([i["pe1_w"], i["pe1_b"][None, :]], 0)
    wp["w2a_bc"] = bc(i["pe2_w"][0:G, 0])
    wp["w2b_bc"] = bc(i["pe2_w"][G:2 * G, 0])
    wp["w2a2_bc"] = bc(i["lpe_w"][0:G, 0])
    wp["w2b2_bc"] = bc(i["lpe_w"][G:2 * G, 0])
    wp["etw"] = i["et_w"]
    wp["etb_row"] = i["et_b"][None, :]
    wp["lpnw"] = i["lpn_w"]
    wp["lpnb_row"] = i["lpn_b"][None, :]
    wp["tw"] = i["t_w"]
    wp["tb_row"] = i["t_b"][None, :]
    for t in range(T_steps):
        wp[f"wA{t}_bc"] = bc(i["r_logit_w"][t, 0:G, 0])
        wp[f"wB{t}_bc"] = bc(i["r_logit_w"][t, G:2 * G, 0])
        wp[f"rpw{t}"] = i["r_proj_w"][t]
        wp[f"rpb{t}_row"] = i["r_proj_b"][t][None, :]
    packs = [("g0", i["gru0_wih"], i["gru0_whh"], i["gru0_bih"], i["gru0_bhh"]),
             ("g1", i["gru1_wih"], i["gru1_whh"], i["gru1_bih"], i["gru1_bhh"])]
    for t in range(T_steps):
        packs.append((f"gr{t}", i["r_gru_wih"][t], i["r_gru_whh"][t],
                      i["r_gru_bih"][t], i["r_gru_bhh"][t]))
    for nm, wih, whh, bih, bhh in packs:
        wp[nm + "_wih"] = wih
        wp[nm + "_whh"] = whh
        wp[nm + "_brz"] = (bih[0:2 * G] + bhh[0:2 * G])[None, :]
        wp[nm + "_bihn"] = bih[2 * G:3 * G][None, :]
        wp[nm + "_bhhn"] = bhh[2 * G:3 * G][None, :]
    wp = {k: np.ascontiguousarray(v, np.float32) for k, v in wp.items()}
    wp["r_proj_w"] = i["r_proj_w"]
    wp["scalars"] = dict(pe2_b=float(i["pe2_b"][0]),
                         lpe_b=float(i["lpe_b"][0]),
                         r_logit_b=[float(x) for x in i["r_logit_b"][:, 0]])
    wp["t_w"] = i["t_w"]
    return wp


def make_in_maps(pl, inputs, wp, bf16=True):
    iota_bc = np.tile(np.arange(P, dtype=np.float32)[None, :], (P, 1))
    ident = np.eye(P, dtype=np.float32)
    ones_row = np.ones((1, P), np.float32)
    in_maps = []
    for k in range(W):
        m = dict(
            node_feats=np.ascontiguousarray(inputs["node_feats"], np.float32),
            nf_loc=np.ascontiguousarray(pl.nf_loc[k]),
            ef_pad=np.ascontiguousarray(pl.ef_pad[k]),
            srcg=np.ascontiguousarray(pl.srcg[k]),
            hexti=np.ascontiguousarray(pl.hexti[k]),
            dstmod=np.ascontiguousarray(pl.dstmod[k]),
            rankcol=np.ascontiguousarray(pl.rankcol[k]),
            iota_bc=iota_bc, ident=ident, ones_row=ones_row,
            ones_col=np.ones((P, 1), np.float32),
        )
        for name, arr in wp.items():
            if name in ("scalars", "t_w", "r_proj_w"):
                continue
            if bf16 and not name.endswith("_bc"):
                m["w_" + name] = arr.astype(NPBF16)
            else:
                m["w_" + name] = arr
        in_maps.append(m)
    return in_maps


def run(inputs, debug=False, trace=False, B=1000, bf16=True,
        act_leaky=True):
    node_feats = np.asarray(inputs["node_feats"], np.float32)
    edge_feats = np.asarray(inputs["edge_feats"], np.float32)
    src = np.asarray(inputs["src"], np.int64)
    dst = np.asarray(inputs["dst"], np.int64)
    graph_ids = np.asarray(inputs["graph_ids"], np.int64)
    G = inputs["et_w"].shape[0]
    PD = inputs["t_w"].shape[1]

    import time as _time
    _t = _time.time()
    pl = preprocess(node_feats, edge_feats, src, dst, graph_ids, B)
    wp = pack_weights(inputs, pl.F, pl.E, G, PD)
    print(f"[kernel] preprocess {_time.time()-_t:.1f}s NBLK={pl.NBLK} "
          f"TT={pl.TT}", flush=True)
    _t = _time.time()
    nc = build_program(pl, wp, debug=debug, bf16=bf16, act_leaky=act_leaky)
    print(f"[kernel] build {_time.time()-_t:.1f}s", flush=True)
    in_maps = make_in_maps(pl, inputs, wp, bf16=bf16)
    _t = _time.time()
    res = bass_utils.run_bass_kernel_spmd(
        nc, in_maps, core_ids=list(range(W)), trace=trace)
    print(f"[kernel] compile+run {_time.time()-_t:.1f}s", flush=True)

    out = np.zeros((B, PD), np.float32)
    # padded-rank row (empty-graph dynamics) — same on every core
    empty_row = res.results[0]["out"][P - 1]
    out[:] = empty_row
    for k in range(W):
        ch = pl.chunks[k]
        if len(ch):
            out[ch] = res.results[k]["out"][0:len(ch)]
    return out, res, pl


def kernel(**inputs):
    out, _, _ = run(inputs, debug=False, trace=False, B=1000)
    # match reference output dtype (float32)
    return out
